# revision 64
# baseline (speedup 1.0000x reference)
"""DAGNN forward on 8 Trainium2 NeuronCores.

Computation: a[:, :512] = x; for node i in topological (index) order:
a[:, i] = tanh(b[i] + sum_j W[i, j] * a[:, j]); y = a[:, 1536:2048].

Strategy (v2):
- Data-parallel over batch: 8 cores x 256 rows each. Activations stored
  transposed on-chip: aT[node, batch].
- Host computes DAG levels, reorders nodes by level (stable), and packs
  nodes into LEVEL-ALIGNED chunks of <=128 nodes (no level spans a chunk
  boundary) -> exactly one tanh round per level (89 rounds), the serial
  floor for this DAG (any consecutive-chunk partition has sum of
  in-chunk depths >= global depth).
- Batch is split into S=2 halves with fully independent aT/psum tiles;
  the two per-level chains interleave on PE/ACT so the scalar engine
  stays saturated: round ~= tanhA + tanhB ~= 650ns instead of
  tanh(256) + matmul + sync latency ~= 1us.
- Per chunk: off-diagonal source blocks (prev chunks -> this chunk) are
  bulk matmuls scheduled as fillers during earlier rounds; the boundary
  block (prev chunk) and in-chunk per-level gather blocks (columns
  masked to the level, zero rows for not-yet-final sources) ride the
  critical chain.
- Every weight block is zero-padded to 128x128 so all matmuls share one
  PE tile configuration, and a budgeted stream of full-array dummy
  matmuls keeps the tensor engine continuously busy: both are needed to
  hold the PE at its full 2.4GHz p-state (idle gaps or tile reconfigs
  drop it to 0.65-1.2GHz and cost ~140ns on every chain matmul).
- All weight DMAs are issued in the prologue into persistent SBUF tiles
  (no WAR gating of the in-order DMA rings); output DMAs go via gpsimd
  mid-kernel (SP-issued DMAs would stall tile-framework semaphore
  traffic) and via SP+ACT hardware DGE for the last two chunks.
"""

import sys

for _p in ("/opt/trn_rl_repo",):
    if _p not in sys.path:
        sys.path.append(_p)

import numpy as np

N_NODES = 2048
N_IN = 512
N_OUT = 512
DEG = 32
BATCH = 2048
NCORES = 8
BCORE = BATCH // NCORES  # 256
NCH_IN = N_IN // 128  # 4 input chunks of 128
NSPLIT = 2  # batch halves per core (independent interleaved chains)
HB = BCORE // NSPLIT  # 128
# PE-warming dummy matmuls: must use the full 128x128 array — low-
# utilization matmuls don't draw enough power to hold the tensor engine
# at its full p-state (the DVFS ramp tracks real load, not just "busy").
DUMF = 128  # free size of dummies (~53ns at full p-state, 107 at mid)
DUMK = 128  # contraction rows of dummies
DUM_PRE = 30  # prologue dummies (~3us at mid p-state completes the ramp)
# per half-round PE budget after the chain matmul, in full-p-state ns.
# Slightly ABOVE the steady half-period: if PE ever idles >~100ns before
# a chain matmul, its p-state resets (chain mm 163->370ns) and the slow
# phase self-reinforces -- the kernel is bistable (~82.5us vs ~97us) when
# undersubscribed. Oversubscribing pins the fast mode; the early rounds
# use a smaller budget because pre-ramp matmuls cost ~2x.
SLOT_BUDGET = 310
SLOT_BUDGET_EARLY = 180
EARLY_ROUNDS = 5
FILL_COST = 56  # full-speed cost of a real off-diag filler (free=128)
DUM_COST = 56  # full-speed cost of a dummy


def _prep(edge_src, edge_dst, edge_w, b):
    """Level-sort the DAG, pack level-aligned chunks and weight blocks."""
    edge_src = np.asarray(edge_src, dtype=np.int64)
    edge_dst = np.asarray(edge_dst, dtype=np.int64)
    edge_w = np.asarray(edge_w, dtype=np.float32)
    b = np.asarray(b, dtype=np.float32)

    src2 = edge_src.reshape(N_NODES - N_IN, DEG)
    level = np.zeros(N_NODES, np.int64)
    for i in range(N_IN, N_NODES):
        level[i] = level[src2[i - N_IN]].max() + 1
    L = int(level.max())

    comp = np.arange(N_IN, N_NODES)
    order = comp[np.argsort(level[N_IN:], kind="stable")]  # old ids by level
    perm = np.concatenate([np.arange(N_IN), order])  # new -> old
    newpos = np.empty(N_NODES, np.int64)
    newpos[perm] = np.arange(N_NODES)

    # dense transposed weights in new coords: WT[src_new, dst_new]
    WT = np.zeros((N_NODES, N_NODES), np.float32)
    np.add.at(WT, (newpos[edge_src], newpos[edge_dst]), edge_w)

    lev_new = level[perm]  # sorted for computed region

    # level-aligned chunks: pack whole levels while size <= 128
    lev_sizes = [int((lev_new[N_IN:] == l).sum()) for l in range(1, L + 1)]
    chunks = []  # dict(base, sz, groups=[(r0, r1)])
    base = N_IN
    cur_sz = 0
    cur_groups = []
    for l, s in enumerate(lev_sizes, start=1):
        assert s > 0
        if cur_sz + s > 128:
            chunks.append(dict(base=base, sz=cur_sz, groups=cur_groups))
            base += cur_sz
            cur_sz = 0
            cur_groups = []
        cur_groups.append((cur_sz, cur_sz + s))
        cur_sz += s
    chunks.append(dict(base=base, sz=cur_sz, groups=cur_groups))
    C = len(chunks)

    # weight block packing into wflat [128, F]
    # per chunk t, in column order: offdiag (inputs then computed 0..t-2),
    # bd (t-1), gathers (levels 1..L_t-1)
    # Every block is zero-padded to exactly 128x128: uniform contraction
    # (K), out-partition count (M) and PE tile config for every matmul.
    # Mixed K tiles cost ~140ns of PE array reconfiguration per switch on
    # hardware, and variable M (65..128) measurably slows the PE even
    # though the rounded tile config is identical.
    cols = []
    col = 0
    MINW = 128
    for t, ch in enumerate(chunks):
        d0, sz = ch["base"], ch["sz"]
        wsz = max(MINW, sz)
        off = []  # (coloff, width, src_kind, src_idx)
        for c in range(NCH_IN):
            blk = WT[c * 128 : (c + 1) * 128, d0 : d0 + sz]
            if blk.any():
                cols.append(blk)
                off.append((col, wsz, "in", c))
                col += wsz
        for s in range(max(0, t - 1)):
            sb, ssz = chunks[s]["base"], chunks[s]["sz"]
            blk = WT[sb : sb + ssz, d0 : d0 + sz]
            if blk.any():
                cols.append(blk)
                off.append((col, wsz, "ch", s))
                col += wsz
        bd = None
        if t > 0:
            sb, ssz = chunks[t - 1]["base"], chunks[t - 1]["sz"]
            blk = WT[sb : sb + ssz, d0 : d0 + sz]
            if blk.any():
                cols.append(blk)
                bd = (col, wsz)
                col += wsz
        gth = []  # per level index >=1: (coloff, width) or None
        D = WT[d0 : d0 + sz, d0 : d0 + sz]
        for gi, (r0, r1) in enumerate(ch["groups"]):
            if gi == 0:
                continue
            w = max(MINW, r1)
            blk = np.zeros((r0, w), np.float32)
            blk[:, r0:r1] = D[:r0, r0:r1]
            if blk.any():
                cols.append(blk)
                gth.append((col, w))
                col += w
            else:
                gth.append(None)
        ch["off"] = off
        ch["bd"] = bd
        ch["gth"] = gth
        ch["w1"] = col
    F = col
    wflat = np.zeros((128, F), np.float32)
    c = 0
    for blk in cols:
        k, w = blk.shape
        wflat[:k, c : c + w] = blk
        c += max(MINW, w)
    assert c == F, (c, F)

    # chunk col ranges
    w0 = 0
    for ch in chunks:
        ch["w0"] = w0
        w0 = ch["w1"]

    bias_pack = np.zeros((128, C), np.float32)
    for t, ch in enumerate(chunks):
        bias_pack[: ch["sz"], t] = b[perm][ch["base"] : ch["base"] + ch["sz"]]

    out_rows = newpos[np.arange(N_NODES - N_OUT, N_NODES)] - N_IN

    return dict(
        perm=perm,
        newpos=newpos,
        chunks=chunks,
        wflat=wflat,
        bias=bias_pack,
        out_rows=out_rows,
        n_rounds=sum(len(ch["groups"]) for ch in chunks),
    )


def _emulate(prep, xT):
    """Numpy emulation of the exact block scheme (per core). xT: [512, B]."""
    B = xT.shape[1]
    wflat = prep["wflat"]
    chunks = prep["chunks"]
    aT = np.zeros((N_NODES, B), np.float32)
    aT[:N_IN] = xT
    bias = prep["bias"]
    for t, ch in enumerate(chunks):
        d0, sz = ch["base"], ch["sz"]
        wsz = psum_w = 128
        psum = np.zeros((psum_w, B), np.float32)
        for coloff, w, kind, s in ch["off"]:
            blk = wflat[:, coloff : coloff + w]
            if kind == "in":
                rows = aT[s * 128 : s * 128 + 128]
            else:
                k = chunks[s]["sz"]
                rows = np.zeros((128, B), np.float32)
                rows[:k] = aT[chunks[s]["base"] : chunks[s]["base"] + k]
            psum[:w] += blk.T @ rows
        if ch["bd"] is not None:
            coloff, w = ch["bd"]
            blk = wflat[:, coloff : coloff + w]
            k = chunks[t - 1]["sz"]
            rows = np.zeros((128, B), np.float32)
            rows[:k] = aT[chunks[t - 1]["base"] : chunks[t - 1]["base"] + k]
            psum[:w] += blk.T @ rows
        for gi, (r0, r1) in enumerate(ch["groups"]):
            if gi > 0 and ch["gth"][gi - 1] is not None:
                coloff, w = ch["gth"][gi - 1]
                blk = wflat[:, coloff : coloff + w]
                rows = np.zeros((128, B), np.float32)
                rows[:sz] = aT[d0 : d0 + sz]  # rows >= r0 hit zero weights
                psum[:w] += blk.T @ rows
            # idempotent full-row tanh rewrite (matches kernel)
            aT[d0 : d0 + sz] = np.tanh(psum[:sz] + bias[:sz, t : t + 1])
    return aT[N_IN:]  # [1536, B]


def _build_program(prep):
    """Build the Bass/Tile program (identical for all 8 cores)."""
    import concourse.bacc as bacc
    import concourse.tile as tile
    from concourse import mybir

    f32 = mybir.dt.float32
    f16 = mybir.dt.float16
    nc = bacc.Bacc(
        "TRN2",
        target_bir_lowering=False,
        debug=False,
        enable_asserts=False,
        num_devices=NCORES,
    )
    chunks = prep["chunks"]
    C = len(chunks)
    wflat = prep["wflat"]
    F = wflat.shape[1]
    NH = NSPLIT

    xT_d = nc.dram_tensor("xT", [128, NCH_IN * BCORE], f16, kind="ExternalInput").ap()
    w_d = nc.dram_tensor("wflat", [128, F], f16, kind="ExternalInput").ap()
    b_d = nc.dram_tensor("bias", [128, C], f32, kind="ExternalInput").ap()
    out_d = nc.dram_tensor(
        "outT", [N_NODES - N_IN, BCORE], f16, kind="ExternalOutput"
    ).ap()

    with tile.TileContext(nc) as tc:
        with (
            tc.tile_pool(name="aT", bufs=1) as aT_pool,
            tc.tile_pool(name="small", bufs=1) as small_pool,
            tc.tile_pool(name="psum", bufs=3 * NH, space="PSUM") as psum_pool,
            tc.tile_pool(name="dpsum", bufs=1, space="PSUM") as dpsum_pool,
        ):
            # persistent tiles
            xin = aT_pool.tile([128, NCH_IN * BCORE], f16, tag="xin", name="xin")
            aT = [
                [
                    aT_pool.tile([128, HB], f16, tag=f"aT{t}h{h}", name=f"aT{t}h{h}")
                    for h in range(NH)
                ]
                for t in range(C)
            ]
            bias_t = small_pool.tile([128, C], f32, tag="bias")
            scratch = small_pool.tile([128, 1], f32, tag="scratch")
            # dummy matmul operand + sink: a stream of dummy matmuls keeps
            # the PE pipeline continuously busy, which (a) ramps the tensor
            # engine to its full p-state (2.4 GHz after 3us of continuous
            # execution; idle gaps reset it to 0.65 GHz) and (b) hides the
            # SBUF pipeline-fill latency of the critical chain matmuls.
            dummyw = small_pool.tile([128, 128], f16, tag="dummyw")
            dummy_ps = dpsum_pool.tile([128, DUMF], f32, tag="dps")

            def emit_dummy(n):
                for _ in range(n):
                    nc.tensor.matmul(
                        dummy_ps[:, :],
                        dummyw[:DUMK, :],
                        dummyw[:DUMK, :DUMF],
                        start=True,
                        stop=True,
                    )

            def xin_view(c, h):
                return xin[:, c * BCORE + h * HB : c * BCORE + (h + 1) * HB]

            # W tiles are persistent (SBUF comfortably fits all ~50KB/
            # partition of weights) and every weight DMA is issued in the
            # prologue: descriptors then stream without WAR gating, so the
            # DMA queues drain continuously and the interleaved output DMAs
            # are not stuck behind stalled weight descriptors.
            w_tiles = [
                aT_pool.tile(
                    [128, ch["w1"] - ch["w0"]], f16, tag=f"w{t}", name=f"w{t}"
                )
                for t, ch in enumerate(chunks)
            ]

            def wdma(t, split_off=False):
                ch = chunks[t]
                wid = ch["w1"] - ch["w0"]
                if split_off and ch["off"]:
                    # first transfer covers only the off-diag blocks so the
                    # prologue matmuls don't wait for the bd/gather columns
                    last = ch["off"][-1]
                    split = last[0] + last[1] - ch["w0"]
                    nc.sync.dma_start(
                        out=w_tiles[t][:, :split],
                        in_=w_d[:, ch["w0"] : ch["w0"] + split],
                    )
                    if split < wid:
                        # first two gather blocks get their own early
                        # completion event (round 2's dependency); the rest
                        # follows after bias
                        mid = min(split + 2 * 128, wid)
                        nc.sync.dma_start(
                            out=w_tiles[t][:, split:mid],
                            in_=w_d[:, ch["w0"] + split : ch["w0"] + mid],
                        )
                        nc.sync.dma_start(out=bias_t[:], in_=b_d[:])
                        if mid < wid:
                            nc.sync.dma_start(
                                out=w_tiles[t][:, mid:],
                                in_=w_d[:, ch["w0"] + mid : ch["w1"]],
                            )
                    return
                nc.sync.dma_start(
                    out=w_tiles[t][:], in_=w_d[:, ch["w0"] : ch["w1"]]
                )

            def wslice(t, coloff, w):
                a = coloff - chunks[t]["w0"]
                return w_tiles[t][:, a : a + w]

            # psum bookkeeping
            psum_t = [[None] * NH for _ in range(C)]
            started = [[False] * NH for _ in range(C)]
            n_mms = []
            for t, ch in enumerate(chunks):
                n = len(ch["off"]) + (1 if ch["bd"] else 0)
                n += sum(1 for g in ch["gth"] if g is not None)
                n_mms.append(n)
            mm_count = [[0] * NH for _ in range(C)]

            def emit_mm(t, coloff, w, rhs_tile, h):
                if psum_t[t][h] is None:
                    psum_t[t][h] = psum_pool.tile(
                        [128, HB], f32, tag="psum", name=f"ps{t}h{h}"
                    )
                first = mm_count[t][h] == 0
                last = mm_count[t][h] == n_mms[t] - 1
                nc.tensor.matmul(
                    psum_t[t][h][:w, :],
                    wslice(t, coloff, w),
                    rhs_tile[:, :],
                    start=first,
                    stop=last,
                )
                mm_count[t][h] += 1

            # filler queue: (dst_t, h, coloff, k, src_kind, src_idx)
            fq = []
            enq_done = set()

            def enqueue_fillers(t):
                # called at start of chunk t: dsts t+1, t+2 with ready srcs
                for d in (t + 1, t + 2):
                    if d >= C:
                        continue
                    for coloff, w, kind, s in chunks[d]["off"]:
                        if kind == "ch" and s > t - 1:
                            continue
                        key = (d, coloff)
                        if key in enq_done:
                            continue
                        enq_done.add(key)
                        for h in range(NH):
                            fq.append((d, h, coloff, w, kind, s))

            def pop_fillers(n, dst_max=None):
                done = 0
                while fq and done < n:
                    if dst_max is not None and fq[0][0] > dst_max:
                        break
                    d, h, coloff, w, kind, s = fq.pop(0)
                    src = xin_view(s, h) if kind == "in" else aT[s][h]
                    emit_mm(d, coloff, w, src, h)
                    done += 1
                return done

            # ---- prologue ----
            # tanh table preload on garbage scratch (no DMA dependency)
            nc.scalar.activation(
                scratch[:], scratch[:], mybir.ActivationFunctionType.Tanh
            )
            nc.vector.memset(dummyw[:], 0.0)
            # zero the never-written tail rows of each computed-chunk tile:
            # full-128 contractions read them (against zero weight rows), and
            # NaN garbage would poison the matmul (0 * NaN = NaN)
            for t, ch in enumerate(chunks):
                if ch["sz"] < 128:
                    for h in range(NH):
                        nc.vector.memset(aT[t][h][:, :], 0.0)
            nc.sync.dma_start(out=xin[:], in_=xT_d[:])
            wdma(0, split_off=True)  # also issues the bias DMA mid-split
            for t in range(1, C):
                wdma(t)
            # start the PE ramp while the prologue DMAs stream
            emit_dummy(DUM_PRE)

            # chunk 0 off-diag (inputs) for both halves
            for h in range(NH):
                for coloff, w, kind, s in chunks[0]["off"]:
                    emit_mm(0, coloff, w, xin_view(s, h), h)

            # ---- rounds ----
            round_idx = 0
            for t, ch in enumerate(chunks):
                enqueue_fillers(t)
                sz = ch["sz"]
                ngroups = len(ch["groups"])
                for gi, (r0, r1) in enumerate(ch["groups"]):
                    last_round = gi == ngroups - 1
                    round_idx += 1
                    for h in range(NH):
                        if gi == 0:
                            if ch["bd"] is not None:
                                coloff, w = ch["bd"]
                                emit_mm(t, coloff, w, aT[t - 1][h], h)
                        else:
                            g = ch["gth"][gi - 1]
                            if g is not None:
                                coloff, w = g
                                emit_mm(t, coloff, w, aT[t][h], h)
                        # full-row idempotent tanh (rows of later levels get
                        # garbage, rewritten by their own round)
                        nc.scalar.activation(
                            aT[t][h][:sz, :],
                            psum_t[t][h][:sz, :],
                            mybir.ActivationFunctionType.Tanh,
                            bias=bias_t[:sz, t : t + 1],
                        )
                        n_fill = pop_fillers(2 if t < 9 else 4)
                        budget = (
                            SLOT_BUDGET_EARLY
                            if round_idx < EARLY_ROUNDS
                            else SLOT_BUDGET
                        )
                        rem = budget - n_fill * FILL_COST
                        emit_dummy(max(0, rem // DUM_COST))
                # drain fillers targeting chunk t+1 before its rounds start
                pop_fillers(len(fq), dst_max=t + 1)
                # output DMA. Mid-kernel chunks go via the idle gpsimd
                # engine (SP-issued out-DMAs stall the round-critical
                # semaphore traffic on the sync queue); the final two chunks
                # use SP and ACT hardware DGE in parallel for a short tail.
                g0 = ch["base"] - N_IN
                for h in range(NH):
                    if t == C - 1:
                        # final chunks: spread halves across engines so all
                        # tail descriptor generations run in parallel
                        dma_eng = nc.scalar if h == 0 else nc.sync
                    elif t == C - 2:
                        dma_eng = nc.sync if h == 0 else nc.gpsimd
                    else:
                        dma_eng = nc.gpsimd
                    dma_eng.dma_start(
                        out=out_d[g0 : g0 + sz, h * HB : (h + 1) * HB],
                        in_=aT[t][h][:sz, :],
                    )

    nc.compile()
    return nc


def _make_in_maps(prep, x):
    x = np.asarray(x, dtype=np.float32)
    wflat16 = prep["wflat"].astype(np.float16)
    bias = prep["bias"]
    in_maps = []
    for r in range(NCORES):
        xr = x[r * BCORE : (r + 1) * BCORE]  # [256, 512]
        xT2 = (
            xr.T.reshape(NCH_IN, 128, BCORE)
            .transpose(1, 0, 2)
            .reshape(128, NCH_IN * BCORE)
        )
        in_maps.append(
            {
                "xT": np.ascontiguousarray(xT2).astype(np.float16),
                "wflat": wflat16,
                "bias": bias,
            }
        )
    return in_maps


def _assemble(prep, results):
    out_rows = prep["out_rows"]
    y = np.empty((BATCH, N_OUT), np.float32)
    for r in range(NCORES):
        outT = results[r]["outT"].astype(np.float32)  # [1536, 256]
        y[r * BCORE : (r + 1) * BCORE, :] = outT[out_rows, :].T
    return y


def kernel(x, edge_w, b, edge_src, edge_dst, n_out, _trace=False):
    n_out = int(n_out)
    assert n_out == N_OUT, f"hardcoded for n_out={N_OUT}, got {n_out}"
    x = np.asarray(x, dtype=np.float32)
    assert x.shape == (BATCH, N_IN)

    from concourse.bass_utils import run_bass_kernel_spmd

    prep = _prep(edge_src, edge_dst, edge_w, b)
    nc = _build_program(prep)
    in_maps = _make_in_maps(prep, x)
    res = run_bass_kernel_spmd(
        nc, in_maps, core_ids=list(range(NCORES)), trace=_trace
    )
    y = _assemble(prep, res.results)
    if _trace:
        kernel._last_exec_time_ns = res.exec_time_ns
        kernel._last_results = res
    return y


if __name__ == "__main__":
    # host-side emulation check against the jax reference
    sys.path.insert(0, "/root/problem")
    import os

    os.environ.setdefault("JAX_PLATFORMS", "cpu")

    inputs = {
        k: np.load(f"/tmp/ref_{k}.npy")
        for k in ("x", "edge_w", "b", "edge_src", "edge_dst")
    }
    expected = np.load("/tmp/ref_out.npy")
    prep = _prep(
        inputs["edge_src"], inputs["edge_dst"], inputs["edge_w"], inputs["b"]
    )
    print(
        f"chunks={len(prep['chunks'])} rounds={prep['n_rounds']} "
        f"F={prep['wflat'].shape[1]}"
    )
    xT = inputs["x"][:8].T.astype(np.float32)  # tiny batch slice
    aT = _emulate(prep, xT)
    got = aT[prep["out_rows"], :].T
    err = np.abs(got - expected[:8]).max()
    rel = err / np.abs(expected[:8]).max()
    print(f"emulation absmax err {err:.3e}  rel {rel:.3e}")
